# revision 49
# baseline (speedup 1.0000x reference)
"""Additive (Bahdanau) attention on 8 Trainium2 NeuronCores.

reference math (per batch b):
    qp = query @ Wq                          [Q, H]
    kp = key @ Wk                            [K, H]
    scores[q,k] = sum_h v[h] * tanh(qp[q,h] + kp[k,h])
    attention = softmax(scores, axis=k)      [Q, K]
    context = attention @ value              [Q, VD]
    returns (context, attention)

Sharding: fully data-parallel, core c handles batch b = c//2 and query rows
qh*256..qh*256+256 (qh = c%2). Softmax is over K which is kept whole per
core, so no collectives are needed.

Per-core device algorithm (h=128 lives on partitions):
  Matrix inputs are cast to bf16 on the host (halves the DMA bytes,
  full-rate PE; the projections accumulate in fp32 PSUM so the tanh
  inputs stay accurate).  DVE pre-adds qp[q] (per-partition scalar) onto
  kpT into [128, 8x1024] fp32 sum tiles; one big ACT tanh per 8 queries
  emits bf16 (ACT is the bottleneck engine: Q*K*H/128 lanes / 1.2GHz =
  218us floor).  PE reduces over h with v as the bf16 moving operand and
  each tanh tile as the stationary operand, writing column q of scoresT
  [k,q] PSUM tiles.  scores are bounded by sum|v| (~9) so softmax needs
  no max-subtraction.  exp on ACT (tanh and exp share one table set ->
  no table switches), PE transposes scoresT to [q,k], a second exp pass
  with accum_out yields the row sums, DVE reciprocal + per-partition
  scales produce attention; the bf16 context matmul runs on PE straight
  from the exp and value tiles.
"""

import os
import sys

import numpy as np

for p in ("/opt/trn_rl_repo",):
    if p not in sys.path and os.path.isdir(p):
        sys.path.insert(0, p)

B, Q, K, QD, KD, VD, H = 4, 512, 1024, 512, 512, 512, 128
NCORES = 8
QS = Q // 2  # query rows per core

_NC_CACHE = None


def _build_nc(reps=1):
    from contextlib import ExitStack

    import concourse.tile as tile
    from concourse import bacc, mybir
    from concourse.masks import make_identity

    f32 = mybir.dt.float32
    bf16 = mybir.dt.bfloat16
    AF = mybir.ActivationFunctionType

    nc = bacc.Bacc(
        "TRN2",
        target_bir_lowering=False,
        debug=False,
        enable_asserts=True,
        num_devices=NCORES,
    )

    qT = nc.declare_dram_parameter("qT", [QD, QS], bf16, isOutput=False)
    kT = nc.declare_dram_parameter("kT", [KD, K], bf16, isOutput=False)
    val = nc.declare_dram_parameter("val", [K, VD], bf16, isOutput=False)
    wq = nc.declare_dram_parameter("wq", [QD, H], bf16, isOutput=False)
    wk = nc.declare_dram_parameter("wk", [KD, H], bf16, isOutput=False)
    vv = nc.declare_dram_parameter("vv", [H, 1], bf16, isOutput=False)
    ctx_o = nc.declare_dram_parameter("ctx", [QS, VD], f32, isOutput=True)
    att_o = nc.declare_dram_parameter("att", [QS, K], f32, isOutput=True)

    DQ = QD // 128  # 4 contraction chunks for the projections
    KC = K // 128  # 8 key chunks

    with tile.TileContext(nc) as tc, ExitStack() as ctx:
        if reps > 1:
            # wall-clock benchmarking only: repeat the whole body on-device
            ctx.enter_context(tc.For_i(0, reps, 1))
        const = ctx.enter_context(tc.tile_pool(name="const", bufs=1))

        wk_sb = const.tile([128, DQ, H], bf16)
        wq_sb = const.tile([128, DQ, H], bf16)
        v_bf = const.tile([128, 1], bf16)
        ident_bf = const.tile([128, 128], bf16)
        kpT_sb = const.tile([128, K], f32)
        qpT_sb = const.tile([128, QS], f32)
        expT_bf = const.tile([128, KC, QS], bf16)

        # preload the exp_and_others ACT table at t~0 (no data deps) so the
        # first real tanh doesn't pay the table-load on the critical path
        scratch = const.tile([128, 1], f32)
        nc.vector.memset(scratch[:], 0.0)
        nc.scalar.activation(scratch[:], scratch[:], AF.Tanh)

        # ---- staging pools. `staging` (kT/qT) is released right after the
        # projections so the main-loop pools' SBUF only waits on that;
        # `valstage` sits lower on the stack and releases later, so the val
        # load stays off the critical path (val is consumed directly as
        # float32r by the context matmul -- no cast needed). ----
        valstage = ctx.enter_context(tc.tile_pool(name="valstage", bufs=1))
        val_sb = valstage.tile([128, KC, VD], bf16)
        with tc.tile_pool(name="staging", bufs=1) as staging:
            kT_sb = staging.tile([128, DQ, K], bf16)
            qT_sb = staging.tile([128, DQ, QS], bf16)
            # One multi-chunk DMA per tensor (HWDGE pays ~0.6us of issue per
            # DMA instruction, so fewer/bigger transfers win), split over the
            # SP and ACT queues. kT is two DMAs (k-halves) so the first
            # projection half can start sooner; val rides the slow POOL
            # SW-DGE queue since it isn't needed until the first epilogue.
            kT_r = kT.ap().rearrange("(i p) k -> p i k", p=128)
            qT_r = qT.ap().rearrange("(i p) x -> p i x", p=128)
            wk_r = wk.ap().rearrange("(i p) h -> p i h", p=128)
            wq_r = wq.ap().rearrange("(i p) h -> p i h", p=128)
            val_r = val.ap().rearrange("(i p) d -> p i d", p=128)
            # The HWDGE round-robins the SP/ACT queues, so interleave the
            # issue order to make the shared DMA engines serve the kp-h0
            # chain (kT-h0, wk) first, the qp chain next, kT-h1 after, and
            # v (only needed by the first matvec) last.
            nc.sync.dma_start(out=kT_sb[:, :, 0:512], in_=kT_r[:, :, 0:512])
            nc.scalar.dma_start(out=wk_sb[:, :, :], in_=wk_r)
            nc.sync.dma_start(out=qT_sb[:, :, :], in_=qT_r)
            nc.scalar.dma_start(out=wq_sb[:, :, :], in_=wq_r)
            nc.sync.dma_start(out=kT_sb[:, :, 512:1024], in_=kT_r[:, :, 512:1024])
            nc.scalar.dma_start(out=v_bf[:], in_=vv[:])
            # val in per-chunk pieces on the POOL SW-DGE queue: its ~1us
            # per-DMA issue cost self-throttles val so it doesn't crowd the
            # critical kT/qT transfers off the shared DMA engines
            for i in range(KC):
                nc.gpsimd.dma_start(out=val_sb[:, i, :], in_=val_r[:, i, :])
            make_identity(nc, ident_bf[:])

            # ---- projections: kpT [h, K], qpT [h, QS] (bf16 inputs from the
            # host: full-rate PE, half the DMA bytes; fp32 PSUM accumulate) ----
            # PSUM->SBUF copies go on ACT (idle during warmup; keeps the DVE
            # stream free for the first adds). Order kp-h0, qp, kp-h1: the
            # first group's adds need kp-h0 + qp first.
            with tc.tile_pool(name="proj_psum", bufs=3, space="PSUM") as proj_psum:
                pt0 = proj_psum.tile([128, 512], f32, name="pt0", tag="pt")
                for d in range(DQ):
                    nc.tensor.matmul(
                        pt0[:],
                        wk_sb[:, d, :],
                        kT_sb[:, d, 0:512],
                        start=(d == 0),
                        stop=(d == DQ - 1),
                    )
                nc.scalar.copy(out=kpT_sb[:, 0:512], in_=pt0[:])

                ptq = proj_psum.tile([128, QS], f32, name="ptq", tag="pt")
                for d in range(DQ):
                    nc.tensor.matmul(
                        ptq[:],
                        wq_sb[:, d, :],
                        qT_sb[:, d, :],
                        start=(d == 0),
                        stop=(d == DQ - 1),
                    )
                nc.scalar.copy(out=qpT_sb[:], in_=ptq[:])

                pt1 = proj_psum.tile([128, 512], f32, name="pt1", tag="pt")
                for d in range(DQ):
                    nc.tensor.matmul(
                        pt1[:],
                        wk_sb[:, d, :],
                        kT_sb[:, d, 512:1024],
                        start=(d == 0),
                        stop=(d == DQ - 1),
                    )
                nc.scalar.copy(out=kpT_sb[:, 512:1024], in_=pt1[:])

        # ---- main loop: add + tanh + h-reduction into scoresT [k, q],
        #      epilogue for each 128-query block interleaved after its
        #      columns complete ----
        GQ = 8  # queries per tanh batch (amortizes ACT per-instr overhead)
        with (
            tc.tile_pool(name="scores", bufs=1, space="PSUM") as scores_pool,
            tc.tile_pool(name="sums", bufs=2) as sum_pool,
            tc.tile_pool(name="tanh", bufs=2) as tanh_pool,
            tc.tile_pool(name="s_psum", bufs=1, space="PSUM") as s_pool,
            tc.tile_pool(name="ctx_psum", bufs=2, space="PSUM") as ctx_pool,
            tc.tile_pool(name="small", bufs=8) as small_pool,
            tc.tile_pool(name="outs", bufs=4) as out_pool,
        ):
            # scoresT per q-block: 2 bank-tiles of 4 chunks x 128 cols
            sc = [
                [
                    scores_pool.tile(
                        [128, 4, 128], f32, name=f"sc{qb}_{i}", tag=f"sc{qb}_{i}"
                    )
                    for i in range(2)
                ]
                for qb in range(2)
            ]

            def epilogue(qb):
                # scoresT chunk tiles for this q-block -> exp on ACT (fp32,
                # feeds the f32r context matmul) + fp32 copy -> PE transpose
                # -> second exp with row-sum accumulation -> 1/sum scales ->
                # DMA out
                if qb == 0:
                    # mid-kernel: bank-tile granularity amortizes the ACT
                    # per-instruction overhead (latency has slack here)
                    for i in range(2):
                        nc.scalar.activation(
                            expT_bf[:, 4 * i : 4 * (i + 1), 0:128],
                            sc[qb][i][:, :, :],
                            AF.Exp,
                        )
                else:
                    # tail: per-chunk so each exp starts right after its
                    # chunk's last matvec column lands
                    for c in range(KC):
                        nc.scalar.activation(
                            expT_bf[:, c, 128 * qb : 128 * (qb + 1)],
                            sc[qb][c // 4][:, c % 4, :],
                            AF.Exp,
                        )
                # context matmul needs only expT/val (bf16): runs on PE
                # while ACT does the second exp pass
                # transpose the exp values themselves (bf16, exact on PE)
                # into [q, k]; row sums + normalization run on DVE straight
                # from PSUM -- no fp32 score copies, no second exp pass
                e_ps = s_pool.tile([128, KC, 128], bf16, name="e_ps", tag="e_ps")
                for c in range(KC):
                    nc.tensor.transpose(
                        e_ps[:, c, :],
                        expT_bf[:, c, 128 * qb : 128 * (qb + 1)],
                        ident_bf[:],
                    )
                sums0 = small_pool.tile([128, 1], f32, name="sums0", tag="sums0")
                sums1 = small_pool.tile([128, 1], f32, name="sums1", tag="sums1")
                nc.vector.tensor_reduce(
                    out=sums0[:], in_=e_ps[:, 0:4, :],
                    axis=mybir.AxisListType.XY, op=mybir.AluOpType.add,
                )
                nc.vector.tensor_reduce(
                    out=sums1[:], in_=e_ps[:, 4:8, :],
                    axis=mybir.AxisListType.XY, op=mybir.AluOpType.add,
                )
                r = small_pool.tile([128, 1], f32, name="r", tag="r")
                nc.vector.tensor_add(r[:], sums0[:], sums1[:])
                nc.vector.reciprocal(r[:], r[:])

                cps = ctx_pool.tile([128, VD], f32, name="cps", tag="cps")
                for c in range(KC):
                    nc.tensor.matmul(
                        cps[:],
                        expT_bf[:, c, 128 * qb : 128 * (qb + 1)],
                        val_sb[:, c, :],
                        start=(c == 0),
                        stop=(c == KC - 1),
                    )

                att_sb = out_pool.tile([128, K], f32, name="att_sb", tag="att_sb")
                for half in range(2):
                    sl = slice(512 * half, 512 * (half + 1))
                    nc.vector.tensor_scalar_mul(
                        att_sb[:, sl],
                        e_ps[:, 4 * half : 4 * (half + 1), :],
                        r[:],
                    )
                    nc.sync.dma_start(
                        out=att_o[128 * qb : 128 * (qb + 1), sl],
                        in_=att_sb[:, sl],
                    )
                ctx_sb = out_pool.tile([128, VD], f32, name="ctx_sb", tag="ctx_sb")
                nc.vector.tensor_scalar_mul(ctx_sb[:], cps[:], r[:])
                nc.scalar.dma_start(
                    out=ctx_o[128 * qb : 128 * (qb + 1), :], in_=ctx_sb[:]
                )

            groups_per_block = 128 // GQ
            for g in range(QS // GQ):
                tq = tanh_pool.tile([128, GQ, K], bf16, name="tq", tag="tq")
                if g == 0:
                    # ramp-up: group 0 uses the ACT bias path straight from
                    # kpT (per-partition bias = qp[q]) in k-halves -- no DVE
                    # dependency, so ACT starts as soon as kpT is copied and
                    # DVE is free to pre-add group 1 immediately
                    for j in range(GQ):
                        nc.scalar.activation(
                            tq[:, j, :],
                            kpT_sb[:],
                            AF.Tanh,
                            bias=qpT_sb[:, j : j + 1],
                        )
                else:
                    last_of_block = (g + 1) % groups_per_block == 0
                    sum_t = sum_pool.tile(
                        [128, GQ, K], f32, name="sum_t", tag="sum_t"
                    )
                    for j in range(GQ):
                        q = GQ * g + j
                        nc.vector.tensor_scalar_add(
                            sum_t[:, j, :], kpT_sb[:], qpT_sb[:, q : q + 1]
                        )
                    if last_of_block:
                        # split so the epilogue-feeding matvecs start half a
                        # tanh earlier (the tail hangs off them)
                        nc.scalar.activation(
                            tq[:, 0 : GQ // 2, :], sum_t[:, 0 : GQ // 2, :],
                            AF.Tanh,
                        )
                        nc.scalar.activation(
                            tq[:, GQ // 2 :, :], sum_t[:, GQ // 2 :, :],
                            AF.Tanh,
                        )
                    else:
                        nc.scalar.activation(
                            tq[:, :, :], sum_t[:, :, :], AF.Tanh
                        )
                # last group before an epilogue runs c-major (in two j-waves
                # matching the split tanh) so each chunk's exp can start as
                # soon as that chunk's columns are in
                last_of_block = (g + 1) % groups_per_block == 0
                order = (
                    [
                        (j, c)
                        for wave in (range(0, GQ // 2), range(GQ // 2, GQ))
                        for c in range(KC)
                        for j in wave
                    ]
                    if last_of_block
                    else [(j, c) for j in range(GQ) for c in range(KC)]
                )
                for j, c in order:
                    q = GQ * g + j
                    qb, qo = divmod(q, 128)
                    nc.tensor.matmul(
                        sc[qb][c // 4][:, c % 4, qo : qo + 1],
                        tq[:, j, 128 * c : 128 * (c + 1)],
                        v_bf[:],
                        start=True,
                        stop=True,
                    )
                if last_of_block:
                    epilogue((g + 1) // groups_per_block - 1)

    nc.compile()
    return nc


def get_nc(reps=1):
    global _NC_CACHE
    if reps != 1:
        return _build_nc(reps=reps)
    if _NC_CACHE is None:
        _NC_CACHE = _build_nc()
    return _NC_CACHE


def make_in_maps(query, key, value, Wq, Wk, v):
    import ml_dtypes

    bf = ml_dtypes.bfloat16
    query = np.asarray(query, dtype=np.float32)
    key = np.asarray(key, dtype=np.float32)
    value = np.asarray(value, dtype=np.float32)
    Wq = np.ascontiguousarray(np.asarray(Wq, dtype=np.float32).astype(bf))
    Wk = np.ascontiguousarray(np.asarray(Wk, dtype=np.float32).astype(bf))
    vv = np.ascontiguousarray(
        np.asarray(v, dtype=np.float32).reshape(H, 1).astype(bf)
    )

    in_maps = []
    for c in range(NCORES):
        b, qh = divmod(c, 2)
        in_maps.append(
            {
                "qT": np.ascontiguousarray(
                    query[b, qh * QS : (qh + 1) * QS, :].T.astype(bf)
                ),
                "kT": np.ascontiguousarray(key[b].T.astype(bf)),
                "val": np.ascontiguousarray(value[b].astype(bf)),
                "wq": Wq,
                "wk": Wk,
                "vv": vv,
            }
        )
    return in_maps


def assemble(results):
    context = np.empty((B, Q, VD), np.float32)
    attention = np.empty((B, Q, K), np.float32)
    for c in range(NCORES):
        b, qh = divmod(c, 2)
        context[b, qh * QS : (qh + 1) * QS, :] = results[c]["ctx"]
        attention[b, qh * QS : (qh + 1) * QS, :] = results[c]["att"]
    return context, attention


def kernel(query, key, value, Wq, Wk, v):
    from concourse.bass_utils import run_bass_kernel_spmd

    nc = get_nc()
    in_maps = make_in_maps(query, key, value, Wq, Wk, v)
    res = run_bass_kernel_spmd(nc, in_maps, core_ids=list(range(NCORES))).results
    return assemble(res)


# revision 54
# speedup vs baseline: 1.0091x; 1.0091x over previous
"""Additive (Bahdanau) attention on 8 Trainium2 NeuronCores.

reference math (per batch b):
    qp = query @ Wq                          [Q, H]
    kp = key @ Wk                            [K, H]
    scores[q,k] = sum_h v[h] * tanh(qp[q,h] + kp[k,h])
    attention = softmax(scores, axis=k)      [Q, K]
    context = attention @ value              [Q, VD]
    returns (context, attention)

Sharding: fully data-parallel, core c handles batch b = c//2 and query rows
qh*256..qh*256+256 (qh = c%2). Softmax is over K which is kept whole per
core, so no collectives are needed.

Per-core device algorithm (h=128 lives on partitions):
  Matrix inputs are cast to bf16 on the host (halves the DMA bytes,
  full-rate PE; the projections accumulate in fp32 PSUM so the tanh
  inputs stay accurate).  DVE pre-adds qp[q] (per-partition scalar) onto
  kpT into [128, 8x1024] fp32 sum tiles; one big ACT tanh per 8 queries
  emits bf16 (ACT is the bottleneck engine: Q*K*H/128 lanes / 1.2GHz =
  218us floor).  PE reduces over h with v as the bf16 moving operand and
  each tanh tile as the stationary operand, writing column q of scoresT
  [k,q] PSUM tiles.  scores are bounded by sum|v| (~9) so softmax needs
  no max-subtraction.  exp on ACT (tanh and exp share one table set ->
  no table switches), PE transposes the bf16 exp values to [q,k], DVE
  reduces the row sums + reciprocal, per-partition scales produce
  attention; the bf16 context matmul runs on PE straight from the exp
  and value tiles, with the same 1/sum applied to its PSUM result.
"""

import os
import sys

import numpy as np

for p in ("/opt/trn_rl_repo",):
    if p not in sys.path and os.path.isdir(p):
        sys.path.insert(0, p)

B, Q, K, QD, KD, VD, H = 4, 512, 1024, 512, 512, 512, 128
NCORES = 8
QS = Q // 2  # query rows per core

_NC_CACHE = None


def _build_nc(reps=1):
    from contextlib import ExitStack

    import concourse.tile as tile
    from concourse import bacc, mybir
    from concourse.masks import make_identity

    f32 = mybir.dt.float32
    bf16 = mybir.dt.bfloat16
    AF = mybir.ActivationFunctionType

    nc = bacc.Bacc(
        "TRN2",
        target_bir_lowering=False,
        debug=False,
        enable_asserts=True,
        num_devices=NCORES,
    )

    qT = nc.declare_dram_parameter("qT", [QD, QS], bf16, isOutput=False)
    kT = nc.declare_dram_parameter("kT", [KD, K], bf16, isOutput=False)
    val = nc.declare_dram_parameter("val", [K, VD], bf16, isOutput=False)
    wq = nc.declare_dram_parameter("wq", [QD, H], bf16, isOutput=False)
    wk = nc.declare_dram_parameter("wk", [KD, H], bf16, isOutput=False)
    vv = nc.declare_dram_parameter("vv", [H, 1], bf16, isOutput=False)
    ctx_o = nc.declare_dram_parameter("ctx", [QS, VD], f32, isOutput=True)
    att_o = nc.declare_dram_parameter("att", [QS, K], f32, isOutput=True)

    DQ = QD // 128  # 4 contraction chunks for the projections
    KC = K // 128  # 8 key chunks

    with tile.TileContext(nc) as tc, ExitStack() as ctx:
        if reps > 1:
            # wall-clock benchmarking only: repeat the whole body on-device
            ctx.enter_context(tc.For_i(0, reps, 1))
        const = ctx.enter_context(tc.tile_pool(name="const", bufs=1))

        wk_sb = const.tile([128, DQ, H], bf16)
        wq_sb = const.tile([128, DQ, H], bf16)
        v_bf = const.tile([128, 1], bf16)
        ident_bf = const.tile([128, 128], bf16)
        kpT_sb = const.tile([128, K], f32)
        qpT_sb = const.tile([128, QS], f32)
        expT_bf = const.tile([128, KC, QS], bf16)

        # preload the exp_and_others ACT table at t~0 (no data deps) so the
        # first real tanh doesn't pay the table-load on the critical path
        scratch = const.tile([128, 1], f32)
        nc.vector.memset(scratch[:], 0.0)
        nc.scalar.activation(scratch[:], scratch[:], AF.Tanh)

        # ---- staging pools. `staging` (kT/qT) is released right after the
        # projections so the main-loop pools' SBUF only waits on that;
        # `valstage` sits lower on the stack and releases later, so the val
        # load stays off the critical path (val is consumed directly by
        # the bf16 context matmul -- no cast needed). ----
        valstage = ctx.enter_context(tc.tile_pool(name="valstage", bufs=1))
        val_sb = valstage.tile([128, KC, VD], bf16)
        with tc.tile_pool(name="staging", bufs=1) as staging:
            kT_sb = staging.tile([128, DQ, K], bf16)
            qT_sb = staging.tile([128, DQ, QS], bf16)
            # One multi-chunk DMA per tensor (HWDGE pays ~0.6us of issue per
            # DMA instruction, so fewer/bigger transfers win), split over the
            # SP and ACT queues. kT is two DMAs (k-halves) so the first
            # projection half can start sooner; val rides the slow POOL
            # SW-DGE queue since it isn't needed until the first epilogue.
            kT_r = kT.ap().rearrange("(i p) k -> p i k", p=128)
            qT_r = qT.ap().rearrange("(i p) x -> p i x", p=128)
            wk_r = wk.ap().rearrange("(i p) h -> p i h", p=128)
            wq_r = wq.ap().rearrange("(i p) h -> p i h", p=128)
            val_r = val.ap().rearrange("(i p) d -> p i d", p=128)
            # The HWDGE round-robins the SP/ACT queues, so interleave the
            # issue order to make the shared DMA engines serve the kp-h0
            # chain (kT-h0, wk) first, the qp chain next, kT-h1 after, and
            # v (only needed by the first matvec) last.
            nc.sync.dma_start(out=kT_sb[:, :, 0:512], in_=kT_r[:, :, 0:512])
            nc.scalar.dma_start(out=wk_sb[:, :, :], in_=wk_r)
            nc.sync.dma_start(out=qT_sb[:, :, :], in_=qT_r)
            nc.scalar.dma_start(out=wq_sb[:, :, :], in_=wq_r)
            nc.sync.dma_start(out=kT_sb[:, :, 512:1024], in_=kT_r[:, :, 512:1024])
            nc.scalar.dma_start(out=v_bf[:], in_=vv[:])
            # val in per-chunk pieces on the POOL SW-DGE queue: its ~1us
            # per-DMA issue cost self-throttles val so it doesn't crowd the
            # critical kT/qT transfers off the shared DMA engines
            for i in range(KC):
                nc.gpsimd.dma_start(out=val_sb[:, i, :], in_=val_r[:, i, :])
            make_identity(nc, ident_bf[:])

            # ---- projections: kpT [h, K], qpT [h, QS] (bf16 inputs from the
            # host: full-rate PE, half the DMA bytes; fp32 PSUM accumulate) ----
            # PSUM->SBUF copies go on ACT (idle during warmup; keeps the DVE
            # stream free for the first adds). Order kp-h0, qp, kp-h1: the
            # first group's adds need kp-h0 + qp first.
            with tc.tile_pool(name="proj_psum", bufs=3, space="PSUM") as proj_psum:
                pt0 = proj_psum.tile([128, 512], f32, name="pt0", tag="pt")
                for d in range(DQ):
                    nc.tensor.matmul(
                        pt0[:],
                        wk_sb[:, d, :],
                        kT_sb[:, d, 0:512],
                        start=(d == 0),
                        stop=(d == DQ - 1),
                    )
                nc.scalar.copy(out=kpT_sb[:, 0:512], in_=pt0[:])

                ptq = proj_psum.tile([128, QS], f32, name="ptq", tag="pt")
                for d in range(DQ):
                    nc.tensor.matmul(
                        ptq[:],
                        wq_sb[:, d, :],
                        qT_sb[:, d, :],
                        start=(d == 0),
                        stop=(d == DQ - 1),
                    )
                nc.scalar.copy(out=qpT_sb[:], in_=ptq[:])

                pt1 = proj_psum.tile([128, 512], f32, name="pt1", tag="pt")
                for d in range(DQ):
                    nc.tensor.matmul(
                        pt1[:],
                        wk_sb[:, d, :],
                        kT_sb[:, d, 512:1024],
                        start=(d == 0),
                        stop=(d == DQ - 1),
                    )
                nc.scalar.copy(out=kpT_sb[:, 512:1024], in_=pt1[:])

        # ---- main loop: add + tanh + h-reduction into scoresT [k, q],
        #      epilogue for each 128-query block interleaved after its
        #      columns complete ----
        GQ = 8  # queries per tanh batch (amortizes ACT per-instr overhead)
        with (
            tc.tile_pool(name="scores", bufs=1, space="PSUM") as scores_pool,
            tc.tile_pool(name="sums", bufs=2) as sum_pool,
            tc.tile_pool(name="tanh", bufs=2) as tanh_pool,
            tc.tile_pool(name="s_psum", bufs=1, space="PSUM") as s_pool,
            tc.tile_pool(name="ctx_psum", bufs=2, space="PSUM") as ctx_pool,
            tc.tile_pool(name="small", bufs=8) as small_pool,
            tc.tile_pool(name="outs", bufs=4) as out_pool,
        ):
            # scoresT per q-block: 2 bank-tiles of 4 chunks x 128 cols
            sc = [
                [
                    scores_pool.tile(
                        [128, 4, 128], f32, name=f"sc{qb}_{i}", tag=f"sc{qb}_{i}"
                    )
                    for i in range(2)
                ]
                for qb in range(2)
            ]

            def epilogue(qb):
                # scoresT chunk tiles for this q-block -> exp on ACT (fp32,
                # feeds the f32r context matmul) + fp32 copy -> PE transpose
                # -> second exp with row-sum accumulation -> 1/sum scales ->
                # DMA out
                if qb == 0:
                    # mid-kernel: bank-tile granularity amortizes the ACT
                    # per-instruction overhead (latency has slack here)
                    for i in range(2):
                        nc.scalar.activation(
                            expT_bf[:, 4 * i : 4 * (i + 1), 0:128],
                            sc[qb][i][:, :, :],
                            AF.Exp,
                        )
                else:
                    # tail: per-chunk so each exp starts right after its
                    # chunk's last matvec column lands
                    for c in range(KC):
                        nc.scalar.activation(
                            expT_bf[:, c, 128 * qb : 128 * (qb + 1)],
                            sc[qb][c // 4][:, c % 4, :],
                            AF.Exp,
                        )
                # context matmul needs only expT/val (bf16): runs on PE
                # while ACT does the second exp pass
                # transpose the exp values themselves (bf16, exact on PE)
                # into [q, k]; row sums + normalization run on DVE straight
                # from PSUM -- no fp32 score copies, no second exp pass
                e_ps = s_pool.tile([128, KC, 128], bf16, name="e_ps", tag="e_ps")
                for c in range(KC):
                    nc.tensor.transpose(
                        e_ps[:, c, :],
                        expT_bf[:, c, 128 * qb : 128 * (qb + 1)],
                        ident_bf[:],
                    )
                sums0 = small_pool.tile([128, 1], f32, name="sums0", tag="sums0")
                sums1 = small_pool.tile([128, 1], f32, name="sums1", tag="sums1")
                nc.vector.tensor_reduce(
                    out=sums0[:], in_=e_ps[:, 0:4, :],
                    axis=mybir.AxisListType.XY, op=mybir.AluOpType.add,
                )
                nc.vector.tensor_reduce(
                    out=sums1[:], in_=e_ps[:, 4:8, :],
                    axis=mybir.AxisListType.XY, op=mybir.AluOpType.add,
                )
                r = small_pool.tile([128, 1], f32, name="r", tag="r")
                nc.vector.tensor_add(r[:], sums0[:], sums1[:])
                nc.vector.reciprocal(r[:], r[:])

                cps = ctx_pool.tile([128, VD], f32, name="cps", tag="cps")
                for c in range(KC):
                    nc.tensor.matmul(
                        cps[:],
                        expT_bf[:, c, 128 * qb : 128 * (qb + 1)],
                        val_sb[:, c, :],
                        start=(c == 0),
                        stop=(c == KC - 1),
                    )

                att_sb = out_pool.tile([128, K], f32, name="att_sb", tag="att_sb")
                for half in range(2):
                    sl = slice(512 * half, 512 * (half + 1))
                    nc.vector.tensor_scalar_mul(
                        att_sb[:, sl],
                        e_ps[:, 4 * half : 4 * (half + 1), :],
                        r[:],
                    )
                    nc.sync.dma_start(
                        out=att_o[128 * qb : 128 * (qb + 1), sl],
                        in_=att_sb[:, sl],
                    )
                ctx_sb = out_pool.tile([128, VD], f32, name="ctx_sb", tag="ctx_sb")
                nc.vector.tensor_scalar_mul(ctx_sb[:], cps[:], r[:])
                nc.scalar.dma_start(
                    out=ctx_o[128 * qb : 128 * (qb + 1), :], in_=ctx_sb[:]
                )

            groups_per_block = 128 // GQ
            for g in range(QS // GQ):
                tq = tanh_pool.tile([128, GQ, K], bf16, name="tq", tag="tq")
                if g == 0:
                    # ramp-up: group 0 uses the ACT bias path straight from
                    # kpT (per-partition bias = qp[q]) in k-halves -- no DVE
                    # dependency, so ACT starts as soon as kpT is copied and
                    # DVE is free to pre-add group 1 immediately
                    for j in range(GQ):
                        nc.scalar.activation(
                            tq[:, j, :],
                            kpT_sb[:],
                            AF.Tanh,
                            bias=qpT_sb[:, j : j + 1],
                        )
                else:
                    last_of_block = (g + 1) % groups_per_block == 0
                    sum_t = sum_pool.tile(
                        [128, GQ, K], f32, name="sum_t", tag="sum_t"
                    )
                    for j in range(GQ):
                        q = GQ * g + j
                        nc.vector.tensor_scalar_add(
                            sum_t[:, j, :], kpT_sb[:], qpT_sb[:, q : q + 1]
                        )
                    if last_of_block:
                        # split so the epilogue-feeding matvecs start half a
                        # tanh earlier (the tail hangs off them)
                        nc.scalar.activation(
                            tq[:, 0 : GQ // 2, :], sum_t[:, 0 : GQ // 2, :],
                            AF.Tanh,
                        )
                        nc.scalar.activation(
                            tq[:, GQ // 2 :, :], sum_t[:, GQ // 2 :, :],
                            AF.Tanh,
                        )
                    else:
                        nc.scalar.activation(
                            tq[:, :, :], sum_t[:, :, :], AF.Tanh
                        )
                # last group before an epilogue runs c-major (in two j-waves
                # matching the split tanh) so each chunk's exp can start as
                # soon as that chunk's columns are in
                last_of_block = (g + 1) % groups_per_block == 0
                order = (
                    [
                        (j, c)
                        for wave in (range(0, GQ // 2), range(GQ // 2, GQ))
                        for c in range(KC)
                        for j in wave
                    ]
                    if last_of_block
                    else [(j, c) for j in range(GQ) for c in range(KC)]
                )
                for j, c in order:
                    q = GQ * g + j
                    qb, qo = divmod(q, 128)
                    nc.tensor.matmul(
                        sc[qb][c // 4][:, c % 4, qo : qo + 1],
                        tq[:, j, 128 * c : 128 * (c + 1)],
                        v_bf[:],
                        start=True,
                        stop=True,
                    )
                if last_of_block:
                    epilogue((g + 1) // groups_per_block - 1)

    nc.compile()
    return nc


def get_nc(reps=1):
    global _NC_CACHE
    if reps != 1:
        return _build_nc(reps=reps)
    if _NC_CACHE is None:
        _NC_CACHE = _build_nc()
    return _NC_CACHE


def make_in_maps(query, key, value, Wq, Wk, v):
    import ml_dtypes

    bf = ml_dtypes.bfloat16
    query = np.asarray(query, dtype=np.float32)
    key = np.asarray(key, dtype=np.float32)
    value = np.asarray(value, dtype=np.float32)
    Wq = np.ascontiguousarray(np.asarray(Wq, dtype=np.float32).astype(bf))
    Wk = np.ascontiguousarray(np.asarray(Wk, dtype=np.float32).astype(bf))
    vv = np.ascontiguousarray(
        np.asarray(v, dtype=np.float32).reshape(H, 1).astype(bf)
    )

    in_maps = []
    for c in range(NCORES):
        b, qh = divmod(c, 2)
        in_maps.append(
            {
                "qT": np.ascontiguousarray(
                    query[b, qh * QS : (qh + 1) * QS, :].T.astype(bf)
                ),
                "kT": np.ascontiguousarray(key[b].T.astype(bf)),
                "val": np.ascontiguousarray(value[b].astype(bf)),
                "wq": Wq,
                "wk": Wk,
                "vv": vv,
            }
        )
    return in_maps


def assemble(results):
    context = np.empty((B, Q, VD), np.float32)
    attention = np.empty((B, Q, K), np.float32)
    for c in range(NCORES):
        b, qh = divmod(c, 2)
        context[b, qh * QS : (qh + 1) * QS, :] = results[c]["ctx"]
        attention[b, qh * QS : (qh + 1) * QS, :] = results[c]["att"]
    return context, attention


def kernel(query, key, value, Wq, Wk, v):
    from concourse.bass_utils import run_bass_kernel_spmd

    nc = get_nc()
    in_maps = make_in_maps(query, key, value, Wq, Wk, v)
    res = run_bass_kernel_spmd(nc, in_maps, core_ids=list(range(NCORES))).results
    return assemble(res)
